# revision 22
# baseline (speedup 1.0000x reference)
"""Trainium2 Bass kernel for nn_DynamicConvolution.

Reference computation (per batch b, T=4096 timesteps, C=512 channels):
    h  = x @ w_in.T + b_in                    # (T, 2C)
    xg = h[:, :C] * sigmoid(h[:, C:])         # GLU -> (T, C)
    w  = softmax((xg @ w_wt.T + b_wt).reshape(T, H, K), axis=-1)
    out[c, t] = sum_k xg[t+k-3, c] * w[t, h(c), k]    # depthwise dynamic conv
    y  = (out + conv_bias) @ w_out.T + b_out

Sharding: data-parallel over batch B=8 -> one batch element per NeuronCore.
Each core runs an identical program on its slice; no collectives.

Per-core dataflow (all matmuls bf16, fp32 accumulation):
  - x is PE-transposed to xT (C-major) to feed mm1 (contraction over C).
  - mm1 produces h token-major; GLU on ACT+DVE -> xg (token-major, bf16).
  - xg is PE-transposed to xgT for the weight-projection matmul.
  - softmax over K on DVE/ACT -> wsm stored [p, j, m] (token-major).
  - The dynamic conv is computed as a banded matmul per (h, time-tile):
    out_h = xg_slab.T @ D, where D[t', t] = w[h, t'-t+3, t] is a 7-diagonal
    band matrix. D is materialized with one gpsimd local_scatter per time
    tile from a pre-shifted copy of the softmax weights (data_all); the
    per-partition scatter indices are host-precomputed constants.
  - Cross-tile band halo is handled by a second tiny matmul (N=4 columns)
    accumulating into the next tile's PSUM.
  - mm_out contracts C (conv output is C-major already) -> y.
"""

import os
import sys

import numpy as np

for _p in ("/opt/trn_rl_repo", os.path.expanduser("~/.axon_site/_ro/trn_rl_repo")):
    if os.path.isdir(_p) and _p not in sys.path:
        sys.path.insert(0, _p)

import concourse.bacc as bacc
import concourse.bass as bass
import concourse.mybir as mybir
import concourse.tile as tile
from concourse.bass_utils import run_bass_kernel_spmd

try:
    import ml_dtypes

    BF16 = np.dtype(ml_dtypes.bfloat16)
except ImportError:  # pragma: no cover
    BF16 = None

T, B, C = 4096, 8, 512
H, K = 8, 7
PAD_L = K // 2
C2 = 2 * C
HK = H * K  # 56
P = 128

F32 = mybir.dt.float32
BF = mybir.dt.bfloat16
I16 = mybir.dt.int16

# Dt tile layout: per h a 136-wide block of "main" band columns, then per h
# an 8-wide block of halo ("prev") columns feeding the next time tile, then
# per h an 8-wide block of halo ("next") columns feeding the previous tile.
MAIN_W = 136
PREV_OFF = H * MAIN_W  # 1088
NEXT_OFF = PREV_OFF + H * 8  # 1152
DT_W = NEXT_OFF + H * 8  # 1216


def ts(i, size):
    return slice(i * size, (i + 1) * size)


def host_scatter_idxs():
    """Scatter index table: data element (p, i, h) -> column of the Dt tile.

    data[p, i*8+h] = wsm[t0 + p + i - 3, 7h + 6 - i]; its band position is
    column j = p + i - 3 of the h'th main block, or (j - 128) of the h'th
    prev block when j >= 128.  j < 0 entries are dropped (-1).
    """
    p = np.arange(P)[:, None, None]
    i = np.arange(K)[None, :, None]
    h = np.arange(H)[None, None, :]
    j = p + i - 3
    main = MAIN_W * h + j
    prev = PREV_OFF + 8 * h + (j - P)
    nxt = NEXT_OFF + 8 * h + (j + 3)
    idx = np.where(j < 0, nxt, np.where(j < P, main, prev))
    return np.ascontiguousarray(idx.reshape(P, K * H).astype(np.int16))


def build_nc(t_len=T, with_bias_in=False, with_bias_wt=False, with_bias_out=False,
             with_conv_bias=False, dbg=False):
    """Build the single-core Bass program (shared by all 8 cores)."""
    NT = t_len // P  # time tiles of 128
    NT4 = t_len // 512  # time tiles of 512 used by mm1/mm_out

    nc = bacc.Bacc()

    from contextlib import ExitStack
    _psum_stack = ExitStack()

    def ctx_enter(cm):
        return _psum_stack.enter_context(cm)

    def ctx_exit():
        _psum_stack.close()

    x_d = nc.declare_dram_parameter("x", [t_len, C], F32, isOutput=False)
    w_inT_d = nc.declare_dram_parameter("w_inT", [P, 4, C2], BF, isOutput=False)
    w_wtT_d = nc.declare_dram_parameter("w_wtT", [P, 4, HK], BF, isOutput=False)
    w_outT_d = nc.declare_dram_parameter("w_outT", [P, 4, C], BF, isOutput=False)
    idxs_d = nc.declare_dram_parameter("idxs", [P, HK], I16, isOutput=False)
    ident32_d = nc.declare_dram_parameter("ident32", [P, P], F32, isOutput=False)
    ident16_d = nc.declare_dram_parameter("ident16", [P, P], BF, isOutput=False)
    if with_bias_in:
        b_in_d = nc.declare_dram_parameter("b_in", [C2], F32, isOutput=False)
    if with_bias_wt:
        b_wt_d = nc.declare_dram_parameter("b_wt", [HK], F32, isOutput=False)
    if with_bias_out:
        b_out_d = nc.declare_dram_parameter("b_out", [C], F32, isOutput=False)
    if with_conv_bias:
        cb4_d = nc.declare_dram_parameter("cb4", [P, 4], F32, isOutput=False)
    y_d = nc.declare_dram_parameter("y", [t_len, C], F32, isOutput=True)
    if dbg:
        NTd = t_len // P
        xg_dbg = nc.declare_dram_parameter("xg_dbg", [P, NTd, C], BF, isOutput=True)
        xgT_dbg = nc.declare_dram_parameter("xgT_dbg", [P, 4, t_len], BF, isOutput=True)
        wsm_dbg = nc.declare_dram_parameter("wsm_dbg", [P, HK, NTd], BF, isOutput=True)
        data_dbg = nc.declare_dram_parameter("data_dbg", [P, NTd, HK], BF, isOutput=True)
        conv_dbg = nc.declare_dram_parameter("conv_dbg", [P, 4, t_len], BF, isOutput=True)
        dt_dbg = nc.declare_dram_parameter("dt_dbg", [P, NTd, DT_W], BF, isOutput=True)

    with tile.TileContext(nc) as tc:
        with (
            tc.tile_pool(name="const", bufs=1) as const,
            tc.tile_pool(name="big", bufs=1) as big,
            tc.tile_pool(name="xin", bufs=3) as xin,
            tc.tile_pool(name="work", bufs=3) as work,
            tc.tile_pool(name="dtp", bufs=3) as dtp,
            tc.tile_pool(name="outp", bufs=3) as outp,
        ):
            # ---- constants ----
            sb_winT = const.tile([P, 4, C2], BF)
            nc.sync.dma_start(sb_winT[:], w_inT_d[:])
            sb_wwtT = const.tile([P, 4, HK], BF)
            nc.sync.dma_start(sb_wwtT[:], w_wtT_d[:])
            sb_woutT = const.tile([P, 4, C], BF)
            nc.sync.dma_start(sb_woutT[:], w_outT_d[:])
            sb_idxs = const.tile([P, HK], I16)
            nc.sync.dma_start(sb_idxs[:], idxs_d[:])
            sb_id32 = const.tile([P, P], F32)
            nc.sync.dma_start(sb_id32[:], ident32_d[:])
            sb_id16 = const.tile([P, P], BF)
            nc.sync.dma_start(sb_id16[:], ident16_d[:])
            if with_bias_in:
                sb_bin = const.tile([P, C2], F32)
                nc.sync.dma_start(sb_bin[:], b_in_d[None, :].to_broadcast((P, C2)))
            if with_bias_wt:
                sb_bwt = const.tile([P, HK], F32)
                nc.sync.dma_start(sb_bwt[:], b_wt_d[None, :].to_broadcast((P, HK)))
            if with_bias_out:
                sb_bout = const.tile([P, C], F32)
                nc.sync.dma_start(sb_bout[:], b_out_d[None, :].to_broadcast((P, C)))
            if with_conv_bias:
                sb_cb4 = const.tile([P, 4], F32)
                nc.sync.dma_start(sb_cb4[:], cb4_d[:])

            # ---- persistent activations ----
            xT = big.tile([P, 4, t_len], BF)       # [c%128, c//128, t]
            xg = big.tile([P, NT, C], BF)          # [t%128, t//128, c]
            xgT = big.tile([P, 4, t_len], BF)      # [c%128, c//128, t]
            conv = big.tile([P, 4, t_len], BF)     # [c%128, c//128, t]
            wsm = big.tile([P, HK, NT], BF)        # [t%128, j, t//128]
            data_tmp = big.tile([P, K, H, NT], BF)
            data_all = big.tile([P, NT, HK], BF)

            # ======== pass 1: x -> xT -> mm1/GLU -> xg -> xgT -> wsm ========
            ps_xt = ctx_enter(tc.tile_pool(name="ps_xt", bufs=2,
                                           space=bass.MemorySpace.PSUM))
            ps_mm1 = ctx_enter(tc.tile_pool(name="ps_mm1", bufs=1,
                                            space=bass.MemorySpace.PSUM))
            ps_xgt = ctx_enter(tc.tile_pool(name="ps_xgt", bufs=2,
                                            space=bass.MemorySpace.PSUM))
            ps_wt = ctx_enter(tc.tile_pool(name="ps_wt", bufs=2,
                                           space=bass.MemorySpace.PSUM))
            for m in range(NT):
                x_t = xin.tile([P, C], F32)
                nc.sync.dma_start(x_t[:], x_d[ts(m, P), :])

                pxT = ps_xt.tile([P, 4, P], F32, tag="pxT")
                for q in range(4):
                    nc.tensor.transpose(pxT[:, q, :], x_t[:, ts(q, P)], sb_id32[:])
                nc.scalar.copy(xT[:, :, ts(m, P)], pxT[:])

            for m in range(NT):
                ps_a = ps_mm1.tile([P, C], F32, tag="ps_a")
                ps_g = ps_mm1.tile([P, C], F32, tag="ps_g")
                for q in range(4):
                    lhs = xT[:, q, ts(m, P)]
                    nc.tensor.matmul(ps_a[:], lhs, sb_winT[:, q, 0:C],
                                     start=(q == 0), stop=(q == 3))
                for q in range(4):
                    lhs = xT[:, q, ts(m, P)]
                    nc.tensor.matmul(ps_g[:], lhs, sb_winT[:, q, C:C2],
                                     start=(q == 0), stop=(q == 3))

                sig = work.tile([P, C], F32, tag="sig")
                if with_bias_in:
                    tmp_g = work.tile([P, C], F32, tag="tmp_g")
                    nc.vector.tensor_add(tmp_g[:], ps_g[:], sb_bin[:, C:C2])
                    nc.scalar.activation(sig[:], tmp_g[:],
                                         mybir.ActivationFunctionType.Sigmoid)
                    tmp_a = work.tile([P, C], F32, tag="tmp_a")
                    nc.vector.tensor_add(tmp_a[:], ps_a[:], sb_bin[:, 0:C])
                    nc.vector.tensor_mul(xg[:, m, :], tmp_a[:], sig[:])
                else:
                    nc.scalar.activation(sig[:], ps_g[:],
                                         mybir.ActivationFunctionType.Sigmoid)
                    nc.vector.tensor_mul(xg[:, m, :], ps_a[:], sig[:])

                # xg -> xgT (PE transpose, bf16)
                pxgT = ps_xgt.tile([P, 4, P], BF, tag="pxgT")
                for q in range(4):
                    nc.tensor.transpose(pxgT[:, q, :], xg[:, m, ts(q, P)], sb_id16[:])
                nc.scalar.copy(xgT[:, :, ts(m, P)], pxgT[:])

                # dynamic weights + softmax over K
                pw = ps_wt.tile([P, HK], F32, tag="pw")
                for q in range(4):
                    nc.tensor.matmul(pw[:], xgT[:, q, ts(m, P)], sb_wwtT[:, q, :],
                                     start=(q == 0), stop=(q == 3))
                logit_src = pw[:]
                if with_bias_wt:
                    logit = work.tile([P, HK], F32, tag="logit")
                    nc.vector.tensor_add(logit[:], pw[:], sb_bwt[:])
                    logit_src = logit[:]
                l3 = logit_src.rearrange("p (h k) -> p h k", k=K)
                mx = work.tile([P, H], F32, tag="mx")
                nc.vector.reduce_max(mx[:], l3, axis=mybir.AxisListType.X)
                e = work.tile([P, HK], F32, tag="e")
                e3 = e[:].rearrange("p (h k) -> p h k", k=K)
                nc.vector.tensor_sub(e3, l3, mx[:, :, None].to_broadcast((P, H, K)))
                nc.scalar.activation(e[:], e[:], mybir.ActivationFunctionType.Exp)
                s = work.tile([P, H], F32, tag="s")
                nc.vector.reduce_sum(s[:], e3, axis=mybir.AxisListType.X)
                r = work.tile([P, H], F32, tag="r")
                nc.vector.reciprocal(r[:], s[:])
                w_dst = wsm[:, :, m].rearrange("p (h k) -> p h k", k=K)
                nc.vector.tensor_mul(w_dst, e3, r[:, :, None].to_broadcast((P, H, K)))

            # ======== shifted weight copies (data_all) ========
            nc.gpsimd.memset(data_tmp[:], 0.0)
            for i in range(K):
                d = i - 3
                rows = wsm[:, 6 - i::K, :]  # [128, H, NT] (j = 7h + 6 - i)
                if d == 0:
                    nc.sync.dma_start(data_tmp[:, i, :, :], rows)
                elif d < 0:
                    nc.sync.dma_start(data_tmp[-d:P, i, :, :], wsm[0:P + d, 6 - i::K, :])
                    if NT > 1:
                        nc.sync.dma_start(data_tmp[0:-d, i, :, 1:NT],
                                          wsm[P + d:P, 6 - i::K, 0:NT - 1])
                else:
                    nc.sync.dma_start(data_tmp[0:P - d, i, :, :], wsm[d:P, 6 - i::K, :])
                    if NT > 1:
                        nc.sync.dma_start(data_tmp[P - d:P, i, :, 0:NT - 1],
                                          wsm[0:d, 6 - i::K, 1:NT])
            # permute [p, i, h, m] -> [p, m, (i, h)]
            da4 = data_all[:].rearrange("p m (i h) -> p m i h", h=H)
            nc.vector.tensor_copy(da4, data_tmp[:].transpose([0, 3, 1, 2]))

            # ======== pass 2: banded-matmul conv + output matmul ========
            ctx_exit()  # release pass-1 PSUM pools
            ps_c = ctx_enter(tc.tile_pool(name="ps_c", bufs=2,
                                          space=bass.MemorySpace.PSUM))
            ps_o = ctx_enter(tc.tile_pool(name="ps_o", bufs=2,
                                          space=bass.MemorySpace.PSUM))
            def scatter_dt(m):
                dt = dtp.tile([P, DT_W], BF, tag="dt")
                nc.gpsimd.local_scatter(dt[:], data_all[:, m, :], sb_idxs[:],
                                        channels=P, num_elems=DT_W, num_idxs=HK)
                if dbg:
                    nc.sync.dma_start(dt_dbg[:, m, :], dt[:])
                return dt

            dt_prev = None
            dt_cur = scatter_dt(0)
            for m in range(NT):
                dt_next = scatter_dt(m + 1) if m + 1 < NT else None
                for cq in range(4):
                    pc = ps_c.tile([P, P], F32, tag="pc")
                    for hh in (2 * cq, 2 * cq + 1):
                        pb = (hh % 2) * 64
                        last = dt_next is None
                        nc.tensor.matmul(pc[pb:pb + 64, :], xg[:, m, ts(hh, 64)],
                                         dt_cur[:, MAIN_W * hh:MAIN_W * hh + P],
                                         start=True, stop=(m == 0 and last),
                                         skip_group_check=True)
                        if m > 0:
                            nc.tensor.matmul(pc[pb:pb + 64, 0:4],
                                             xg[:, m - 1, ts(hh, 64)],
                                             dt_prev[:, PREV_OFF + 8 * hh:PREV_OFF + 8 * hh + 4],
                                             start=False, stop=last,
                                             skip_group_check=True)
                        if dt_next is not None:
                            nc.tensor.matmul(pc[pb:pb + 64, P - 3:P],
                                             xg[:, m + 1, ts(hh, 64)],
                                             dt_next[:, NEXT_OFF + 8 * hh:NEXT_OFF + 8 * hh + 3],
                                             start=False, stop=True,
                                             skip_group_check=True)
                    if with_conv_bias:
                        nc.scalar.add(conv[:, cq, ts(m, P)], pc[:],
                                      sb_cb4[:, cq:cq + 1])
                    else:
                        nc.scalar.copy(conv[:, cq, ts(m, P)], pc[:])
                dt_prev, dt_cur = dt_cur, dt_next

                po = ps_o.tile([P, C], F32, tag="po")
                for q in range(4):
                    nc.tensor.matmul(po[:], conv[:, q, ts(m, P)], sb_woutT[:, q, :],
                                     start=(q == 0), stop=(q == 3))
                out_t = outp.tile([P, C], F32, tag="out_t")
                if with_bias_out:
                    nc.vector.tensor_add(out_t[:], po[:], sb_bout[:])
                else:
                    nc.vector.tensor_copy(out_t[:], po[:])
                nc.sync.dma_start(y_d[ts(m, P), :], out_t[:])

            ctx_exit()  # release pass-2 PSUM pools

            if dbg:
                nc.sync.dma_start(xg_dbg[:], xg[:])
                nc.sync.dma_start(xgT_dbg[:], xgT[:])
                nc.sync.dma_start(wsm_dbg[:], wsm[:])
                nc.sync.dma_start(data_dbg[:], data_all[:])
                nc.sync.dma_start(conv_dbg[:], conv[:])

    nc.compile()
    return nc


def host_inputs(x_b, w_in, b_in, w_wt, b_wt, w_out, b_out, conv_bias,
                with_bias_in, with_bias_wt, with_bias_out, with_conv_bias):
    """Per-core input map from a batch slice + shared weights."""
    def t_pack(w, width):
        # w: [width, C] -> [128, 4, width] bf16 with [p, q, f] = w[f, 128q+p]
        a = np.ascontiguousarray(
            w.T.reshape(4, P, width).transpose(1, 0, 2)).astype(BF16)
        return a

    m = {
        "x": np.ascontiguousarray(x_b, dtype=np.float32),
        "w_inT": t_pack(w_in, C2),
        "w_wtT": t_pack(w_wt, HK),
        "w_outT": t_pack(w_out, C),
        "idxs": host_scatter_idxs(),
        "ident32": np.eye(P, dtype=np.float32),
        "ident16": np.eye(P).astype(BF16),
    }
    if with_bias_in:
        m["b_in"] = np.asarray(b_in, np.float32)
    if with_bias_wt:
        m["b_wt"] = np.asarray(b_wt, np.float32)
    if with_bias_out:
        m["b_out"] = np.asarray(b_out, np.float32)
    if with_conv_bias:
        m["cb4"] = np.ascontiguousarray(
            np.asarray(conv_bias, np.float32).reshape(4, P).T)
    return m


_NC_CACHE = {}


def _get_nc(key):
    if key not in _NC_CACHE:
        _NC_CACHE[key] = build_nc(T, *key)
    return _NC_CACHE[key]


def kernel(x, w_in, b_in, w_wt, b_wt, w_out, b_out, conv_bias, _trace=False):
    x = np.asarray(x)
    flags = (bool(np.any(b_in)), bool(np.any(b_wt)), bool(np.any(b_out)),
             bool(np.any(conv_bias)))
    nc = _get_nc(flags)
    in_maps = [
        host_inputs(x[:, b, :], np.asarray(w_in), b_in, np.asarray(w_wt), b_wt,
                    np.asarray(w_out), b_out, conv_bias, *flags)
        for b in range(B)
    ]
    res = run_bass_kernel_spmd(nc, in_maps, core_ids=list(range(B)),
                               trace=_trace)
    y = np.stack([np.asarray(res.results[b]["y"]) for b in range(B)], axis=1)
    if _trace:
        return y.astype(np.float32), res
    return y.astype(np.float32)
